# revision 6
# baseline (speedup 1.0000x reference)
"""CrossAttentionGate kernel for Trainium2, 8 NeuronCores.

Problem: B=4 batches of single-head spatial cross-attention:
    q = Wq@gate + bq          [B,64,N]   (N = 64*64 = 4096)
    k = Wk@skip + bk          [B,64,N]   (bk drops: softmax shift-invariant)
    v = Wv@skip + bv          [B,256,N]  (bv folded into host-side residual)
    attn = softmax_j(q^T k)   [B,N,N]
    out = gamma * (v @ attn^T) + skip

Sharding: 8 cores = 4 batches x 2 query-halves. Each core computes its
batch's k/v in full and attends for its 2048 query positions.

Layout: transposed-PV flash attention over 256-wide query stripes.
Logits ST[j,i] (fp32 PSUM) -> ACT exp over [128,512] tiles -> P bf16 ->
PV with P chunks stationary / vt moving; vt carries a ones column so
the softmax denominator Z[i] accumulates as output channel 256. Per-partition
1/Z * gamma scaling in a cheap DVE epilogue; output [NI,CS], host
untransposes.

Design notes (HW-measured):
  - All DRAM inputs fp16 (halves DMA-in bytes; precision budget checked:
    fp16 q/k give ~5e-3 abs logit error -> ~0.5% weight error, well inside
    the 2e-2 gate).
  - q_sb/k_sb are [128, N] fp16 with partitions 64:127 zeroed, so ST
    matmuls are full (128,128)-tile-mode ops: no PE array mode switching
    (64-row tiling <-> full mode forces an array drain), and fp16 weights
    are FWL-eligible with standalone LDWEIGHTS pull-ahead (fp32r weights
    self-load serially inside the matmul).
  - Projection PSUM->SBUF copies alternate ACT/DVE so neither engine
    paces the projection phase (was: all on DVE, 24us serial).
  - 256-wide query stripes: p_st tiles are 1 PSUM bank each and p_ot
    accumulators 2 banks per stripe, so double-buffered p_ot (xstripe)
    lets adjacent stripes' accumulators coexist: PV consumers carry
    across stripe boundaries (sw_pipe=12-jg lag) and the PE never
    drains at a stripe seam.
"""

import numpy as np

import concourse.bass as bass
import concourse.tile as tile
from concourse import bacc, mybir
from concourse.bass_utils import run_bass_kernel_spmd

F32 = mybir.dt.float32
F16 = mybir.dt.float16
F8E4 = mybir.dt.float8e4
BF16 = mybir.dt.bfloat16
AF = mybir.ActivationFunctionType
ALU = mybir.AluOpType

B, CG, CS, INTER, H, W = 4, 512, 256, 64, 64, 64
N = H * W
NCORES = 8
NI = N // 2
NJ = N

BEST = dict(sw_pipe=12, p_bufs=12, st_bufs=4, pv_order="ib_u", sw=256,
            xstripe=True)


def _build_program(hw_loop=0, sw_pipe=4, p_bufs=8, st_bufs=2,
                   do_attn=True, no_st=False, fake_p=False,
                   proj_alt=True, st_dtype=F16, exp_batch=2,
                   pv_order="u_ib", st_split=1, proj_split256=False,
                   skipt_dt=F16, proj_bufs=4, sw=512, xstripe=False):
    nc = bacc.Bacc(
        "TRN2", target_bir_lowering=False, debug=False, num_devices=NCORES
    )
    d_gate = nc.dram_tensor("gate", [CG, NI], F16, kind="ExternalInput").ap()
    d_skip = nc.dram_tensor("skip", [CS, NJ], F16, kind="ExternalInput").ap()
    d_skipt = nc.dram_tensor("skipt", [NI, CS], skipt_dt,
                             kind="ExternalInput").ap()
    d_wqt = nc.dram_tensor("wqt", [CG, INTER], F16, kind="ExternalInput").ap()
    d_wkt = nc.dram_tensor("wkt", [CS, INTER], F16, kind="ExternalInput").ap()
    d_wvt = nc.dram_tensor("wvt", [CS, CS], F16, kind="ExternalInput").ap()
    d_bq = nc.dram_tensor("bq", [INTER, 1], F32, kind="ExternalInput").ap()
    d_gam = nc.dram_tensor("gam", [128, 1], F32, kind="ExternalInput").ap()
    d_ones_c = nc.dram_tensor("ones_c", [128, 1], F32, kind="ExternalInput").ap()
    d_out = nc.dram_tensor("out", [NI, CS], F32, kind="ExternalOutput").ap()

    KG = CG // 128   # 4 gate channel tiles
    KS = CS // 128   # 2 skip channel tiles
    JT = NJ // 128   # 32 key tiles
    NT = NI // sw    # query column stripes
    IBN = sw // 128  # i-chunks per stripe
    VT_W = CS + 2    # ones col (softmax denom) + even-width pad

    with tile.TileContext(nc) as tc:
        with (
            tc.tile_pool(name="res", bufs=1) as res,
            tc.tile_pool(name="stream", bufs=4) as stream,
            tc.tile_pool(name="epi", bufs=2) as epi,
        ):
            # ---- load everything ----
            wqt_t = []
            for kk in range(KG):
                t = res.tile([128, INTER], F16, tag=f"wqt{kk}", name=f"wqt{kk}")
                nc.sync.dma_start(t[:], d_wqt[kk * 128:(kk + 1) * 128, :])
                wqt_t.append(t)
            wkt_t = []
            for ss in range(KS):
                t = res.tile([128, INTER], F16, tag=f"wkt{ss}", name=f"wkt{ss}")
                nc.sync.dma_start(t[:], d_wkt[ss * 128:(ss + 1) * 128, :])
                wkt_t.append(t)
            wvt_t = []
            for ss in range(KS):
                t = res.tile([128, CS], F16, tag=f"wvt{ss}", name=f"wvt{ss}")
                nc.sync.dma_start(t[:], d_wvt[ss * 128:(ss + 1) * 128, :])
                wvt_t.append(t)
            bq_t = res.tile([INTER, 1], F32, tag="bq")
            nc.sync.dma_start(bq_t[:], d_bq[:])
            gam_t = res.tile([128, 1], F32, tag="gam")
            nc.sync.dma_start(gam_t[:], d_gam[:])
            ones_c = res.tile([128, 1], F32, tag="ones_c")
            nc.sync.dma_start(ones_c[:], d_ones_c[:])
            skip_t = []
            for ss in range(KS):
                t = res.tile([128, NJ], F16, tag=f"skip{ss}", name=f"skip{ss}")
                nc.sync.dma_start(t[:], d_skip[ss * 128:(ss + 1) * 128, :])
                skip_t.append(t)
            gate_t = []
            for kk in range(KG):
                t = res.tile([128, NI], F16, tag=f"gate{kk}", name=f"gate{kk}")
                nc.sync.dma_start(t[:], d_gate[kk * 128:(kk + 1) * 128, :])
                gate_t.append(t)
            skipt_t = []
            for it in range(NI // 128):
                t = res.tile([128, CS], skipt_dt, tag=f"skipt{it}",
                             name=f"skipt{it}")
                nc.sync.dma_start(t[:], d_skipt[it * 128:(it + 1) * 128, :])
                skipt_t.append(t)

            q_sb = res.tile([128, NI], st_dtype, tag="q_sb")
            k_sb = res.tile([128, NJ], st_dtype, tag="k_sb")
            # zero the dead contraction half once; loop body only writes 0:64
            nc.vector.memset(q_sb[INTER:128, :], 0.0)
            nc.vector.memset(k_sb[INTER:128, :], 0.0)
            vt_sb = [
                res.tile([128, VT_W], BF16, tag=f"vt{jt}", name=f"vt{jt}")
                for jt in range(JT)
            ]
            for jt in range(JT):
                nc.vector.tensor_copy(vt_sb[jt][:, CS:CS + 1], ones_c[:])
                nc.vector.tensor_copy(vt_sb[jt][:, CS + 1:CS + 2], ones_c[:])

            import contextlib
            loop_ctx = tc.For_i(0, hw_loop, 1) if hw_loop else contextlib.nullcontext()
            with loop_ctx:
                # ---- projections ----
                with tc.tile_pool(name="ps_proj", bufs=proj_bufs,
                                  space="PSUM") as ps_proj:
                    # q[d,i] = sum_g WqT[g,d] gate[g,i] + bq
                    psp = 2 if proj_split256 else 1
                    for n in range(NI // 512):
                        pq = ps_proj.tile([INTER, 512], F32, tag="pqk")
                        for kk in range(KG):
                            for hf in range(psp):
                                w = 512 // psp
                                nc.tensor.matmul(
                                    pq[:, hf * w:(hf + 1) * w],
                                    wqt_t[kk][:],
                                    gate_t[kk][:, n * 512 + hf * w:
                                                 n * 512 + (hf + 1) * w],
                                    start=(kk == 0),
                                    stop=(kk == KG - 1),
                                )
                        qdst = q_sb[0:INTER, n * 512:(n + 1) * 512]
                        if proj_alt and n % 2 == 0:
                            nc.scalar.activation(
                                qdst, pq[:], AF.Identity, bias=bq_t[:, 0:1]
                            )
                        else:
                            nc.vector.tensor_scalar(
                                qdst, pq[:], bq_t[:, 0:1], None, op0=ALU.add,
                            )
                    # k[d,j] = sum_s WkT[s,d] skip[s,j]
                    for n in range(NJ // 512):
                        pk = ps_proj.tile([INTER, 512], F32, tag="pqk")
                        for ss in range(KS):
                            for hf in range(psp):
                                w = 512 // psp
                                nc.tensor.matmul(
                                    pk[:, hf * w:(hf + 1) * w],
                                    wkt_t[ss][:],
                                    skip_t[ss][:, n * 512 + hf * w:
                                                 n * 512 + (hf + 1) * w],
                                    start=(ss == 0),
                                    stop=(ss == KS - 1),
                                )
                        kdst = k_sb[0:INTER, n * 512:(n + 1) * 512]
                        if proj_alt and n % 2 == 0:
                            nc.scalar.activation(kdst, pk[:], AF.Copy)
                        else:
                            nc.vector.tensor_copy(kdst, pk[:])
                    # vT[j,c] = sum_s skip[s,j] WvT[s,c]
                    for jt in range(JT):
                        pv = ps_proj.tile([128, CS], F32, tag="pv")
                        for ss in range(KS):
                            nc.tensor.matmul(
                                pv[:],
                                skip_t[ss][:, jt * 128:(jt + 1) * 128],
                                wvt_t[ss][:],
                                start=(ss == 0),
                                stop=(ss == KS - 1),
                            )
                        if proj_alt and jt % 2 == 0:
                            nc.scalar.activation(vt_sb[jt][:, 0:CS], pv[:], AF.Copy)
                        else:
                            nc.vector.tensor_copy(vt_sb[jt][:, 0:CS], pv[:])

                # ---- attention, one 512-wide query stripe at a time ----
                with tc.tile_pool(name="ps_attn", bufs=1, space="PSUM") as ps:
                    pending = []

                    def emit_consumers(item):
                        n, jg, P, p_ot = item
                        pairs = [(u, ib) for u in range(exp_batch)
                                 for ib in range(IBN)]
                        if pv_order == "ib_u":
                            pairs = [(u, ib) for ib in range(IBN)
                                     for u in range(exp_batch)]
                        for u, ib in pairs:
                            jt = jg * exp_batch + u
                            first = jt == 0
                            last = jt == JT - 1
                            nc.tensor.matmul(
                                p_ot[ib][:],
                                P[:, u * sw + ib * 128:
                                  u * sw + (ib + 1) * 128],
                                vt_sb[jt][:],
                                start=first,
                                stop=last,
                            )
                        if jg == JT // exp_batch - 1:
                            epilogue(n, p_ot)

                    def epilogue(n, p_ot):
                        for ib in range(IBN):
                            it = n * IBN + ib
                            rec = epi.tile([128, 1], F32, tag="rec")
                            nc.vector.reciprocal(rec[:], p_ot[ib][:, CS:CS + 1])
                            rg = epi.tile([128, 1], F32, tag="rg")
                            nc.vector.tensor_scalar(
                                rg[:], rec[:], gam_t[:, 0:1], None,
                                op0=ALU.mult,
                            )
                            t0 = epi.tile([128, CS], F32, tag="t0")
                            nc.vector.tensor_scalar(
                                t0[:], p_ot[ib][:, 0:CS], rg[:, 0:1], None,
                                op0=ALU.mult,
                            )
                            out_t = epi.tile([128, CS], F32, tag="out_t")
                            nc.vector.tensor_tensor(
                                out_t[:], t0[:], skipt_t[it][:], op=ALU.add,
                            )
                            nc.sync.dma_start(
                                d_out[it * 128:(it + 1) * 128, :], out_t[:],
                            )

                    for n in range(NT if do_attn else 0):
                        p_ot = [
                            ps.tile([128, VT_W], F32, tag=f"ot{ib}",
                                    name=f"p_ot{ib}",
                                    bufs=2 if xstripe else 1)
                            for ib in range(IBN)
                        ]

                        EB = exp_batch
                        fake_tiles = []
                        if fake_p:
                            p_fk = ps.tile([128, sw * EB], F32, tag="st",
                                           bufs=st_bufs if EB == 2 else 1)
                            nc.tensor.matmul(
                                p_fk[:, 0:sw],
                                k_sb[:, 0:128],
                                q_sb[:, n * sw:(n + 1) * sw],
                                start=True, stop=True,
                            )
                            for b in range(p_bufs):
                                Pf = stream.tile([128, sw * EB], BF16,
                                                 tag="Pf", bufs=p_bufs,
                                                 name=f"Pf{b}")
                                nc.scalar.activation(Pf[:], p_fk[:], AF.Exp)
                                fake_tiles.append(Pf)
                        for jg in range(JT // EB):
                            if not no_st:
                                p_st = ps.tile([128, sw * EB], F32, tag="st",
                                               bufs=st_bufs)
                                for u in range(EB):
                                    jt = jg * EB + u
                                    w = sw // st_split
                                    for hf in range(st_split):
                                        nc.tensor.matmul(
                                            p_st[:, u * sw + hf * w:
                                                 u * sw + (hf + 1) * w],
                                            k_sb[:, jt * 128:(jt + 1) * 128],
                                            q_sb[:, n * sw + hf * w:
                                                 n * sw + (hf + 1) * w],
                                            start=True, stop=True,
                                        )
                            if fake_p:
                                emit_consumers((n, jg, fake_tiles[jg % p_bufs],
                                                p_ot))
                                continue
                            P = stream.tile([128, sw * EB], BF16, tag="P",
                                            bufs=p_bufs)
                            nc.scalar.activation(P[:], p_st[:], AF.Exp)
                            if sw_pipe:
                                pending.append((n, jg, P, p_ot))
                                if len(pending) > sw_pipe:
                                    emit_consumers(pending.pop(0))
                            else:
                                emit_consumers((n, jg, P, p_ot))
                        if not xstripe:
                            for item in pending:
                                emit_consumers(item)
                            pending.clear()
                    for item in pending:
                        emit_consumers(item)
                    pending.clear()
    nc.compile()
    return nc


_PROGRAM_CACHE = None

DMA_IN_BYTES = (
    CG * NI * 2          # gate fp16
    + CS * NJ * 2        # skip fp16
    + NI * CS * 2        # skipt fp16
    + (CG * INTER + CS * INTER + CS * CS) * 2   # weights fp16
    + (INTER + 128 + 128) * 4                   # bq, gam, ones
)


def _cast_skipt(x):
    if BEST.get("skipt_dt", F16) == F16:
        return x.astype(np.float16)
    import ml_dtypes
    return x.astype(ml_dtypes.float8_e4m3)


def make_in_maps(gate, skip, Wq, bq, Wk, bk, Wv, bv, gamma):
    gate = np.ascontiguousarray(np.asarray(gate, dtype=np.float32)).reshape(B, CG, N)
    skip = np.ascontiguousarray(np.asarray(skip, dtype=np.float32)).reshape(B, CS, N)
    Wq = np.asarray(Wq, dtype=np.float32)
    bq = np.asarray(bq, dtype=np.float32)
    Wk = np.asarray(Wk, dtype=np.float32)
    Wv = np.asarray(Wv, dtype=np.float32)
    bv = np.asarray(bv, dtype=np.float32)
    gamma = np.asarray(gamma, dtype=np.float32)

    wqt = np.ascontiguousarray(Wq.T.astype(np.float16))
    wkt = np.ascontiguousarray(Wk.T.astype(np.float16))
    wvt = np.ascontiguousarray(Wv.T.astype(np.float16))
    bq_c = np.ascontiguousarray(bq.reshape(INTER, 1))
    gam = np.full((128, 1), gamma[0], np.float32)
    gbv = (gamma[0] * bv).reshape(CS, 1)
    ones_c = np.ones((128, 1), np.float32)

    gate16 = gate.astype(np.float16)
    skip16 = skip.astype(np.float16)

    in_maps = []
    for core in range(NCORES):
        b, h = divmod(core, 2)
        isl = slice(h * NI, (h + 1) * NI)
        skipr = np.ascontiguousarray(skip[b, :, isl]) + gbv
        in_maps.append(
            {
                "gate": np.ascontiguousarray(gate16[b, :, isl]),
                "skip": skip16[b],
                "skipt": _cast_skipt(np.ascontiguousarray(skipr.T)),
                "wqt": wqt,
                "wkt": wkt,
                "wvt": wvt,
                "bq": bq_c,
                "gam": gam,
                "ones_c": ones_c,
            }
        )
    return in_maps


def kernel(gate, skip, Wq, bq, Wk, bk, Wv, bv, gamma):
    global _PROGRAM_CACHE
    if _PROGRAM_CACHE is None:
        _PROGRAM_CACHE = _build_program(**BEST)
    nc = _PROGRAM_CACHE

    in_maps = make_in_maps(gate, skip, Wq, bq, Wk, bk, Wv, bv, gamma)
    res = run_bass_kernel_spmd(nc, in_maps, list(range(NCORES)))

    out = np.empty((B, CS, N), np.float32)
    for core in range(NCORES):
        b, h = divmod(core, 2)
        o = res.results[core]["out"]
        out[b, :, h * NI:(h + 1) * NI] = o.T
    return out.reshape(B, CS, H, W)


# revision 7
# speedup vs baseline: 1.0289x; 1.0289x over previous
"""CrossAttentionGate kernel for Trainium2, 8 NeuronCores.

Problem: B=4 batches of single-head spatial cross-attention:
    q = Wq@gate + bq          [B,64,N]   (N = 64*64 = 4096)
    k = Wk@skip + bk          [B,64,N]   (bk drops: softmax shift-invariant)
    v = Wv@skip + bv          [B,256,N]  (bv folded into host-side residual)
    attn = softmax_j(q^T k)   [B,N,N]
    out = gamma * (v @ attn^T) + skip

Sharding: 8 cores = 4 batches x 2 query-halves. Each core computes its
batch's k/v in full and attends for its 2048 query positions.

Layout: transposed-PV flash attention over 256-wide query stripes.
Logits ST[j,i] (fp32 PSUM) -> ACT exp over [128,512] tiles -> P bf16 ->
PV with P chunks stationary / vt moving; vt carries a ones column so
the softmax denominator Z[i] accumulates as output channel 256. Per-partition
1/Z * gamma scaling in a cheap DVE epilogue; output [NI,CS], host
untransposes.

Design notes (HW-measured):
  - All DRAM inputs fp16 (halves DMA-in bytes; precision budget checked:
    fp16 q/k give ~5e-3 abs logit error -> ~0.5% weight error, well inside
    the 2e-2 gate).
  - q_sb/k_sb are [128, N] fp16 with partitions 64:127 zeroed, so ST
    matmuls are full (128,128)-tile-mode ops: no PE array mode switching
    (64-row tiling <-> full mode forces an array drain), and fp16 weights
    are FWL-eligible with standalone LDWEIGHTS pull-ahead (fp32r weights
    self-load serially inside the matmul).
  - Projection PSUM->SBUF copies alternate ACT/DVE so neither engine
    paces the projection phase (was: all on DVE, 24us serial).
  - 256-wide query stripes: p_st tiles are 1 PSUM bank each and p_ot
    accumulators 2 banks per stripe, so double-buffered p_ot (xstripe)
    lets adjacent stripes' accumulators coexist: PV consumers carry
    across stripe boundaries (sw_pipe=12-jg lag) and the PE never
    drains at a stripe seam.
"""

import numpy as np

import concourse.bass as bass
import concourse.tile as tile
from concourse import bacc, mybir
from concourse.bass_utils import run_bass_kernel_spmd

F32 = mybir.dt.float32
F16 = mybir.dt.float16
F8E4 = mybir.dt.float8e4
BF16 = mybir.dt.bfloat16
AF = mybir.ActivationFunctionType
ALU = mybir.AluOpType

B, CG, CS, INTER, H, W = 4, 512, 256, 64, 64, 64
N = H * W
NCORES = 8
NI = N // 2
NJ = N

BEST = dict(sw_pipe=12, p_bufs=12, st_bufs=4, pv_order="ib_u", sw=256,
            xstripe=True, skipt_inloop=True)


def _build_program(hw_loop=0, sw_pipe=4, p_bufs=8, st_bufs=2,
                   do_attn=True, no_st=False, fake_p=False,
                   proj_alt=True, st_dtype=F16, exp_batch=2,
                   pv_order="u_ib", st_split=1, proj_split256=False,
                   skipt_dt=F16, proj_bufs=4, sw=512, xstripe=False,
                   skipt_inloop=False):
    nc = bacc.Bacc(
        "TRN2", target_bir_lowering=False, debug=False, num_devices=NCORES
    )
    d_gate = nc.dram_tensor("gate", [CG, NI], F16, kind="ExternalInput").ap()
    d_skip = nc.dram_tensor("skip", [CS, NJ], F16, kind="ExternalInput").ap()
    d_skipt = nc.dram_tensor("skipt", [NI, CS], skipt_dt,
                             kind="ExternalInput").ap()
    d_wqt = nc.dram_tensor("wqt", [CG, INTER], F16, kind="ExternalInput").ap()
    d_wkt = nc.dram_tensor("wkt", [CS, INTER], F16, kind="ExternalInput").ap()
    d_wvt = nc.dram_tensor("wvt", [CS, CS], F16, kind="ExternalInput").ap()
    d_bq = nc.dram_tensor("bq", [INTER, 1], F32, kind="ExternalInput").ap()
    d_gam = nc.dram_tensor("gam", [128, 1], F32, kind="ExternalInput").ap()
    d_ones_c = nc.dram_tensor("ones_c", [128, 1], F32, kind="ExternalInput").ap()
    d_out = nc.dram_tensor("out", [NI, CS], F32, kind="ExternalOutput").ap()

    KG = CG // 128   # 4 gate channel tiles
    KS = CS // 128   # 2 skip channel tiles
    JT = NJ // 128   # 32 key tiles
    NT = NI // sw    # query column stripes
    IBN = sw // 128  # i-chunks per stripe
    VT_W = CS + 2    # ones col (softmax denom) + even-width pad

    with tile.TileContext(nc) as tc:
        with (
            tc.tile_pool(name="res", bufs=1) as res,
            tc.tile_pool(name="stream", bufs=4) as stream,
            tc.tile_pool(name="epi", bufs=2) as epi,
        ):
            # ---- load everything ----
            wqt_t = []
            for kk in range(KG):
                t = res.tile([128, INTER], F16, tag=f"wqt{kk}", name=f"wqt{kk}")
                nc.sync.dma_start(t[:], d_wqt[kk * 128:(kk + 1) * 128, :])
                wqt_t.append(t)
            wkt_t = []
            for ss in range(KS):
                t = res.tile([128, INTER], F16, tag=f"wkt{ss}", name=f"wkt{ss}")
                nc.sync.dma_start(t[:], d_wkt[ss * 128:(ss + 1) * 128, :])
                wkt_t.append(t)
            wvt_t = []
            for ss in range(KS):
                t = res.tile([128, CS], F16, tag=f"wvt{ss}", name=f"wvt{ss}")
                nc.sync.dma_start(t[:], d_wvt[ss * 128:(ss + 1) * 128, :])
                wvt_t.append(t)
            bq_t = res.tile([INTER, 1], F32, tag="bq")
            nc.sync.dma_start(bq_t[:], d_bq[:])
            gam_t = res.tile([128, 1], F32, tag="gam")
            nc.sync.dma_start(gam_t[:], d_gam[:])
            ones_c = res.tile([128, 1], F32, tag="ones_c")
            nc.sync.dma_start(ones_c[:], d_ones_c[:])
            skip_t = []
            for ss in range(KS):
                t = res.tile([128, NJ], F16, tag=f"skip{ss}", name=f"skip{ss}")
                nc.sync.dma_start(t[:], d_skip[ss * 128:(ss + 1) * 128, :])
                skip_t.append(t)
            gate_t = []
            for kk in range(KG):
                t = res.tile([128, NI], F16, tag=f"gate{kk}", name=f"gate{kk}")
                nc.sync.dma_start(t[:], d_gate[kk * 128:(kk + 1) * 128, :])
                gate_t.append(t)
            skipt_t = []
            for it in range(NI // 128):
                t = res.tile([128, CS], skipt_dt, tag=f"skipt{it}",
                             name=f"skipt{it}")
                if not skipt_inloop:
                    nc.sync.dma_start(t[:],
                                      d_skipt[it * 128:(it + 1) * 128, :])
                skipt_t.append(t)

            q_sb = res.tile([128, NI], st_dtype, tag="q_sb")
            k_sb = res.tile([128, NJ], st_dtype, tag="k_sb")
            # zero the dead contraction half once; loop body only writes 0:64
            nc.vector.memset(q_sb[INTER:128, :], 0.0)
            nc.vector.memset(k_sb[INTER:128, :], 0.0)
            vt_sb = [
                res.tile([128, VT_W], BF16, tag=f"vt{jt}", name=f"vt{jt}")
                for jt in range(JT)
            ]
            for jt in range(JT):
                nc.vector.tensor_copy(vt_sb[jt][:, CS:CS + 1], ones_c[:])
                nc.vector.tensor_copy(vt_sb[jt][:, CS + 1:CS + 2], ones_c[:])

            import contextlib
            loop_ctx = tc.For_i(0, hw_loop, 1) if hw_loop else contextlib.nullcontext()
            with loop_ctx:
                if skipt_inloop:
                    for it in range(NI // 128):
                        nc.sync.dma_start(
                            skipt_t[it][:],
                            d_skipt[it * 128:(it + 1) * 128, :])
                # ---- projections ----
                with tc.tile_pool(name="ps_proj", bufs=proj_bufs,
                                  space="PSUM") as ps_proj:
                    # q[d,i] = sum_g WqT[g,d] gate[g,i] + bq
                    psp = 2 if proj_split256 else 1
                    for n in range(NI // 512):
                        pq = ps_proj.tile([INTER, 512], F32, tag="pqk")
                        for kk in range(KG):
                            for hf in range(psp):
                                w = 512 // psp
                                nc.tensor.matmul(
                                    pq[:, hf * w:(hf + 1) * w],
                                    wqt_t[kk][:],
                                    gate_t[kk][:, n * 512 + hf * w:
                                                 n * 512 + (hf + 1) * w],
                                    start=(kk == 0),
                                    stop=(kk == KG - 1),
                                )
                        qdst = q_sb[0:INTER, n * 512:(n + 1) * 512]
                        if proj_alt and n % 2 == 0:
                            nc.scalar.activation(
                                qdst, pq[:], AF.Identity, bias=bq_t[:, 0:1]
                            )
                        else:
                            nc.vector.tensor_scalar(
                                qdst, pq[:], bq_t[:, 0:1], None, op0=ALU.add,
                            )
                    # k[d,j] = sum_s WkT[s,d] skip[s,j]
                    for n in range(NJ // 512):
                        pk = ps_proj.tile([INTER, 512], F32, tag="pqk")
                        for ss in range(KS):
                            for hf in range(psp):
                                w = 512 // psp
                                nc.tensor.matmul(
                                    pk[:, hf * w:(hf + 1) * w],
                                    wkt_t[ss][:],
                                    skip_t[ss][:, n * 512 + hf * w:
                                                 n * 512 + (hf + 1) * w],
                                    start=(ss == 0),
                                    stop=(ss == KS - 1),
                                )
                        kdst = k_sb[0:INTER, n * 512:(n + 1) * 512]
                        if proj_alt and n % 2 == 0:
                            nc.scalar.activation(kdst, pk[:], AF.Copy)
                        else:
                            nc.vector.tensor_copy(kdst, pk[:])
                    # vT[j,c] = sum_s skip[s,j] WvT[s,c]
                    for jt in range(JT):
                        pv = ps_proj.tile([128, CS], F32, tag="pv")
                        for ss in range(KS):
                            nc.tensor.matmul(
                                pv[:],
                                skip_t[ss][:, jt * 128:(jt + 1) * 128],
                                wvt_t[ss][:],
                                start=(ss == 0),
                                stop=(ss == KS - 1),
                            )
                        if proj_alt and jt % 2 == 0:
                            nc.scalar.activation(vt_sb[jt][:, 0:CS], pv[:], AF.Copy)
                        else:
                            nc.vector.tensor_copy(vt_sb[jt][:, 0:CS], pv[:])

                # ---- attention, one 512-wide query stripe at a time ----
                with tc.tile_pool(name="ps_attn", bufs=1, space="PSUM") as ps:
                    pending = []

                    def emit_consumers(item):
                        n, jg, P, p_ot = item
                        pairs = [(u, ib) for u in range(exp_batch)
                                 for ib in range(IBN)]
                        if pv_order == "ib_u":
                            pairs = [(u, ib) for ib in range(IBN)
                                     for u in range(exp_batch)]
                        for u, ib in pairs:
                            jt = jg * exp_batch + u
                            first = jt == 0
                            last = jt == JT - 1
                            nc.tensor.matmul(
                                p_ot[ib][:],
                                P[:, u * sw + ib * 128:
                                  u * sw + (ib + 1) * 128],
                                vt_sb[jt][:],
                                start=first,
                                stop=last,
                            )
                        if jg == JT // exp_batch - 1:
                            epilogue(n, p_ot)

                    def epilogue(n, p_ot):
                        for ib in range(IBN):
                            it = n * IBN + ib
                            rec = epi.tile([128, 1], F32, tag="rec")
                            nc.vector.reciprocal(rec[:], p_ot[ib][:, CS:CS + 1])
                            rg = epi.tile([128, 1], F32, tag="rg")
                            nc.vector.tensor_scalar(
                                rg[:], rec[:], gam_t[:, 0:1], None,
                                op0=ALU.mult,
                            )
                            t0 = epi.tile([128, CS], F32, tag="t0")
                            nc.vector.tensor_scalar(
                                t0[:], p_ot[ib][:, 0:CS], rg[:, 0:1], None,
                                op0=ALU.mult,
                            )
                            out_t = epi.tile([128, CS], F32, tag="out_t")
                            nc.vector.tensor_tensor(
                                out_t[:], t0[:], skipt_t[it][:], op=ALU.add,
                            )
                            nc.sync.dma_start(
                                d_out[it * 128:(it + 1) * 128, :], out_t[:],
                            )

                    for n in range(NT if do_attn else 0):
                        p_ot = [
                            ps.tile([128, VT_W], F32, tag=f"ot{ib}",
                                    name=f"p_ot{ib}",
                                    bufs=2 if xstripe else 1)
                            for ib in range(IBN)
                        ]

                        EB = exp_batch
                        fake_tiles = []
                        if fake_p:
                            p_fk = ps.tile([128, sw * EB], F32, tag="st",
                                           bufs=st_bufs if EB == 2 else 1)
                            nc.tensor.matmul(
                                p_fk[:, 0:sw],
                                k_sb[:, 0:128],
                                q_sb[:, n * sw:(n + 1) * sw],
                                start=True, stop=True,
                            )
                            for b in range(p_bufs):
                                Pf = stream.tile([128, sw * EB], BF16,
                                                 tag="Pf", bufs=p_bufs,
                                                 name=f"Pf{b}")
                                nc.scalar.activation(Pf[:], p_fk[:], AF.Exp)
                                fake_tiles.append(Pf)
                        for jg in range(JT // EB):
                            if not no_st:
                                p_st = ps.tile([128, sw * EB], F32, tag="st",
                                               bufs=st_bufs)
                                for u in range(EB):
                                    jt = jg * EB + u
                                    w = sw // st_split
                                    for hf in range(st_split):
                                        nc.tensor.matmul(
                                            p_st[:, u * sw + hf * w:
                                                 u * sw + (hf + 1) * w],
                                            k_sb[:, jt * 128:(jt + 1) * 128],
                                            q_sb[:, n * sw + hf * w:
                                                 n * sw + (hf + 1) * w],
                                            start=True, stop=True,
                                        )
                            if fake_p:
                                emit_consumers((n, jg, fake_tiles[jg % p_bufs],
                                                p_ot))
                                continue
                            P = stream.tile([128, sw * EB], BF16, tag="P",
                                            bufs=p_bufs)
                            nc.scalar.activation(P[:], p_st[:], AF.Exp)
                            if sw_pipe:
                                pending.append((n, jg, P, p_ot))
                                if len(pending) > sw_pipe:
                                    emit_consumers(pending.pop(0))
                            else:
                                emit_consumers((n, jg, P, p_ot))
                        if not xstripe:
                            for item in pending:
                                emit_consumers(item)
                            pending.clear()
                    for item in pending:
                        emit_consumers(item)
                    pending.clear()
    nc.compile()
    return nc


_PROGRAM_CACHE = None

# skipt (NI*CS fp16) is loaded INSIDE the timed loop (its only consumer,
# the epilogue, is late enough that the DMA fully hides under compute), so
# the slope measures it; the adder covers only the pre-loop loads.
DMA_IN_BYTES = (
    CG * NI * 2          # gate fp16
    + CS * NJ * 2        # skip fp16
    + (CG * INTER + CS * INTER + CS * CS) * 2   # weights fp16
    + (INTER + 128 + 128) * 4                   # bq, gam, ones
)


def _cast_skipt(x):
    if BEST.get("skipt_dt", F16) == F16:
        return x.astype(np.float16)
    import ml_dtypes
    return x.astype(ml_dtypes.float8_e4m3)


def make_in_maps(gate, skip, Wq, bq, Wk, bk, Wv, bv, gamma):
    gate = np.ascontiguousarray(np.asarray(gate, dtype=np.float32)).reshape(B, CG, N)
    skip = np.ascontiguousarray(np.asarray(skip, dtype=np.float32)).reshape(B, CS, N)
    Wq = np.asarray(Wq, dtype=np.float32)
    bq = np.asarray(bq, dtype=np.float32)
    Wk = np.asarray(Wk, dtype=np.float32)
    Wv = np.asarray(Wv, dtype=np.float32)
    bv = np.asarray(bv, dtype=np.float32)
    gamma = np.asarray(gamma, dtype=np.float32)

    wqt = np.ascontiguousarray(Wq.T.astype(np.float16))
    wkt = np.ascontiguousarray(Wk.T.astype(np.float16))
    wvt = np.ascontiguousarray(Wv.T.astype(np.float16))
    bq_c = np.ascontiguousarray(bq.reshape(INTER, 1))
    gam = np.full((128, 1), gamma[0], np.float32)
    gbv = (gamma[0] * bv).reshape(CS, 1)
    ones_c = np.ones((128, 1), np.float32)

    gate16 = gate.astype(np.float16)
    skip16 = skip.astype(np.float16)

    in_maps = []
    for core in range(NCORES):
        b, h = divmod(core, 2)
        isl = slice(h * NI, (h + 1) * NI)
        skipr = np.ascontiguousarray(skip[b, :, isl]) + gbv
        in_maps.append(
            {
                "gate": np.ascontiguousarray(gate16[b, :, isl]),
                "skip": skip16[b],
                "skipt": _cast_skipt(np.ascontiguousarray(skipr.T)),
                "wqt": wqt,
                "wkt": wkt,
                "wvt": wvt,
                "bq": bq_c,
                "gam": gam,
                "ones_c": ones_c,
            }
        )
    return in_maps


def kernel(gate, skip, Wq, bq, Wk, bk, Wv, bv, gamma):
    global _PROGRAM_CACHE
    if _PROGRAM_CACHE is None:
        _PROGRAM_CACHE = _build_program(**BEST)
    nc = _PROGRAM_CACHE

    in_maps = make_in_maps(gate, skip, Wq, bq, Wk, bk, Wv, bv, gamma)
    res = run_bass_kernel_spmd(nc, in_maps, list(range(NCORES)))

    out = np.empty((B, CS, N), np.float32)
    for core in range(NCORES):
        b, h = divmod(core, 2)
        o = res.results[core]["out"]
        out[b, :, h * NI:(h + 1) * NI] = o.T
    return out.reshape(B, CS, H, W)
